# revision 1
# baseline (speedup 1.0000x reference)
"""Causal self-attention (B=4, T=2048, C=1024, H=16, D=64) on 8 TRN2 NeuronCores.

Sharding: core c = (batch b = c//2, head-group hg = c%2 of 8 heads).
Each core computes, for its batch and its 8 heads:
  qkv = x_b @ W_attn[:, cols(hg)] + b_attn[cols(hg)]
  flash causal attention over T=2048 (column-softmax layout)
  partial_out = y_heads @ W_proj[rows(hg), :] (+ b_proj on hg==0 cores)
Host gather: out[b] = partial[2b] + partial[2b+1]  (the tensor-parallel
all-reduce, done at unshard time).

Kernel layout notes:
  - x is PE-transposed on chip to x^T [c, t]; qkv matmuls emit q^T, k^T
    directly in [c', t] layout (stationary = W 128x128 blocks) and v in
    natural [t, c'] layout (stationary = x^T 128x128 blocks).
  - v SBUF tiles are [128, 8*65]: per head 64 value columns plus a ones
    column, so the PV matmul accumulates softmax denominators for free.
  - Attention: S^T blocks [128 keys, 512 queries] = k^T_tile.T @ q^T;
    exp on ACT (scale=1/8 folded in); causal masking of diagonal blocks
    by multiplying with precomputed 0/1 masks (DVE); PV accumulates
    y_aug^T [65, 512] over key blocks; normalize with reciprocal of row
    64 broadcast via a K=1 ones-matmul.
  - All matmuls run as float32r (full PE rate for moving dim >= 256).
"""

import math
import sys

import numpy as np

for _p in ("/opt/trn_rl_repo", "/root/.axon_site/_ro/trn_rl_repo"):
    if _p not in sys.path:
        sys.path.append(_p)

import concourse.bass as bass
import concourse.tile as tile
from concourse import bacc, mybir
from concourse.bass_utils import run_bass_kernel_spmd

dt = mybir.dt
AF = mybir.ActivationFunctionType

B, T, C = 4, 2048, 1024
H, D = 16, 64
N_CORES = 8
S = 512           # per-core section width (8 heads * 64)
TQ = 512          # query chunk
KB = 128          # key block
NTQ = T // TQ     # 4 query chunks
NKB = T // KB     # 16 key blocks
NCC = C // 128    # 8 contraction chunks
NTT = T // 128    # 16 token tiles
HC = H // 2       # 8 local heads


def build_nc(reps: int = 1, stages: str = "abcd"):
    nc = bacc.Bacc("TRN2", target_bir_lowering=False, debug=False,
                   num_devices=N_CORES)

    x_d = nc.dram_tensor("x", [T, C], dt.float32, kind="ExternalInput").ap()
    wqkv_d = nc.dram_tensor("wqkv", [C, 3 * S], dt.float32, kind="ExternalInput").ap()
    bqk_d = nc.dram_tensor("bqk", [128, 8], dt.float32, kind="ExternalInput").ap()
    vb_d = nc.dram_tensor("vbrow", [1, S], dt.float32, kind="ExternalInput").ap()
    wp_d = nc.dram_tensor("wproj", [S, C], dt.float32, kind="ExternalInput").ap()
    pb_d = nc.dram_tensor("pbrow", [1, C], dt.float32, kind="ExternalInput").ap()
    mask_d = nc.dram_tensor("mask", [128, 896], dt.float32, kind="ExternalInput").ap()
    ident_d = nc.dram_tensor("ident", [128, 128], dt.float32, kind="ExternalInput").ap()
    ones_d = nc.dram_tensor("onesm", [128, 128], dt.float32, kind="ExternalInput").ap()

    out_d = nc.dram_tensor("out", [T, C], dt.float32, kind="ExternalOutput").ap()

    with tile.TileContext(nc) as tc:
        if reps == 1:
            _emit(nc, tc, x_d, wqkv_d, bqk_d, vb_d, wp_d, pb_d, mask_d,
                  ident_d, ones_d, out_d, stages)
        else:
            with tc.For_i(0, reps, 1):
                _emit(nc, tc, x_d, wqkv_d, bqk_d, vb_d, wp_d, pb_d, mask_d,
                      ident_d, ones_d, out_d, stages)
    return nc


def _emit(nc, tc, x_d, wqkv_d, bqk_d, vb_d, wp_d, pb_d, mask_d, ident_d,
          ones_d, out_d, stages="abcd"):
    from contextlib import ExitStack

    es = ExitStack()
    with es:
        const = es.enter_context(tc.tile_pool(name="const", bufs=1))
        yT_pool = es.enter_context(tc.tile_pool(name="yT", bufs=1))
        qkv_sb = es.enter_context(tc.tile_pool(name="qkv", bufs=1))

        # --- constants ---
        mask = const.tile([128, 896], dt.float32r, tag="mask")
        nc.gpsimd.dma_start(mask[:], mask_d[:])
        ident = const.tile([128, 128], dt.float32, tag="ident")
        nc.sync.dma_start(ident[:], ident_d[:])
        bqk = const.tile([128, 8], dt.float32, tag="bqk")
        nc.sync.dma_start(bqk[:], bqk_d[:])
        vbrow = const.tile([1, S], dt.float32r, tag="vbrow")
        nc.gpsimd.dma_start(vbrow[:], vb_d[:])
        pbrow = const.tile([1, C], dt.float32r, tag="pbrow")
        nc.gpsimd.dma_start(pbrow[:], pb_d[:])
        onesr = const.tile([128, 128], dt.float32r, tag="onesr")
        nc.gpsimd.dma_start(onesr[:], ones_d[:])
        ones = onesr[0:1, :]

        # --- persistent activations ---
        qT = [qkv_sb.tile([128, T], dt.float32r, tag=f"qT{i}", name=f"qT{i}")
              for i in range(4)]
        kT = [qkv_sb.tile([128, T], dt.float32r, tag=f"kT{i}", name=f"kT{i}")
              for i in range(4)]
        vt = [qkv_sb.tile([128, HC * (D + 1)], dt.float32r, tag=f"v{i}",
                          name=f"v{i}") for i in range(NTT)]

        # ============ Stage A+B: x^T and qkv projections ============
        with ExitStack() as ab:
            wq_sb = ab.enter_context(tc.tile_pool(name="wq", bufs=1))
            xT_sb = ab.enter_context(tc.tile_pool(name="xT", bufs=9))
            xin_sb = ab.enter_context(tc.tile_pool(name="xin", bufs=2))
            ps_tr = ab.enter_context(tc.tile_pool(name="ps_tr", bufs=2, space="PSUM"))
            ps_mm = ab.enter_context(tc.tile_pool(name="ps_mm", bufs=3, space="PSUM"))

            wq = []
            for ccc in range(NCC):
                w = wq_sb.tile([128, 3 * S], dt.float32r, tag=f"w{ccc}")
                nc.gpsimd.dma_start(w[:], wqkv_d[ccc * 128:(ccc + 1) * 128, :])
                wq.append(w)

            TB = 256                  # stage-B token chunk
            for tc4 in range(T // TB):    # 8 chunks of 256 tokens
                xT = [xT_sb.tile([128, TB], dt.float32r, tag="xT", name=f"xT{_i}")
                      for _i in range(NCC)]
                for tt in range(TB // 128):   # 128-token tiles within chunk
                    g = tc4 * (TB // 128) + tt
                    xin = xin_sb.tile([128, C], dt.float32)
                    nc.sync.dma_start(xin[:], x_d[g * 128:(g + 1) * 128, :])
                    for cc in range(NCC):
                        ptr = ps_tr.tile([128, 128], dt.float32)
                        nc.tensor.transpose(
                            ptr[:], xin[:, cc * 128:(cc + 1) * 128], ident[:])
                        nc.scalar.copy(
                            xT[cc][:, tt * 128:(tt + 1) * 128], ptr[:])

                # q^T and k^T sections (8 x 128 columns of W)
                for j in range(8):
                    pmm = ps_mm.tile([128, TB], dt.float32)
                    for cc in range(NCC):
                        nc.tensor.matmul(
                            pmm[:], wq[cc][:, j * 128:(j + 1) * 128], xT[cc][:],
                            start=(cc == 0), stop=(cc == NCC - 1))
                    dest = qT[j] if j < 4 else kT[j - 4]
                    nc.scalar.activation(
                        dest[:, tc4 * TB:(tc4 + 1) * TB], pmm[:],
                        AF.Identity, bias=bqk[:, j:j + 1], scale=1.0)

                # v section (natural layout)
                for tt in range(TB // 128):
                    g = tc4 * (TB // 128) + tt
                    pmm = ps_mm.tile([128, S], dt.float32)
                    for cc in range(NCC):
                        nc.tensor.matmul(
                            pmm[:], xT[cc][:, tt * 128:(tt + 1) * 128],
                            wq[cc][:, 2 * S:3 * S],
                            start=(cc == 0), stop=False)
                    nc.tensor.matmul(pmm[:], ones, vbrow[:],
                                     start=False, stop=True)
                    vtile = vt[g]
                    v3 = vtile[:].rearrange("p (h e) -> p h e", e=D + 1)
                    nc.scalar.copy(
                        v3[:, :, 0:D],
                        pmm[:].rearrange("p (h e) -> p h e", e=D))
                    nc.vector.tensor_copy(v3[:, :, D], onesr[:, 0:HC])

        if "c" not in stages:
            # diagnostic: flush q/k/v so stage B isn't dead code
            for i in range(4):
                nc.sync.dma_start(out_d[i * 128:(i + 1) * 128, :],
                                  qT[i][:, 0:C].bitcast(dt.float32))
                nc.sync.dma_start(out_d[(4 + i) * 128:(5 + i) * 128, :],
                                  kT[i][:, 0:C].bitcast(dt.float32))
            for i in range(8):
                nc.sync.dma_start(
                    out_d[(8 + i) * 128:(9 + i) * 128, 0:HC * (D + 1)],
                    vt[i][:].bitcast(dt.float32))
            return

        yT = [yT_pool.tile([128, T], dt.float32r, tag=f"yT{i}", name=f"yT{i}")
              for i in range(4)]

        # ============ Stage C: causal flash attention ============
        with ExitStack() as at:
            pt_sb = at.enter_context(tc.tile_pool(name="pt", bufs=4))
            rc_sb = at.enter_context(tc.tile_pool(name="rc", bufs=2))
            ps_s = at.enter_context(tc.tile_pool(name="ps_s", bufs=3, space="PSUM"))
            ps_y = at.enter_context(tc.tile_pool(name="ps_y", bufs=2, space="PSUM"))

            # Global software pipeline over all (head, query-chunk, pair)
            # work items: emit S/exp/mask one pair AHEAD of the PV matmuls
            # so PE always has S-work while ACT runs the exp.
            work = []
            for h in range(HC):
                for qc in range(NTQ):
                    npair = (4 * qc + 4) // 2
                    for p in range(npair):
                        work.append((h, qc, p, npair))

            pt_of = {}
            y_of = {}

            def emit_s(idx):
                h, qc, p, npair = work[idx]
                ht, hp = h // 2, (h % 2) * D
                qs = qT[ht][hp:hp + D, qc * TQ:(qc + 1) * TQ]
                s_ps = ps_s.tile([128, 2 * TQ], dt.float32, tag="s", name=f"s{idx}")
                for half in range(2):
                    kb = 2 * p + half
                    nc.tensor.matmul(
                        s_ps[:, half * TQ:(half + 1) * TQ],
                        kT[ht][hp:hp + D, kb * 128:(kb + 1) * 128],
                        qs, start=True, stop=True)
                pt = pt_sb.tile([128, 2 * TQ], dt.float32r, tag="pt",
                                name=f"pt{idx}")
                nc.scalar.activation(pt[:], s_ps[:], AF.Exp,
                                     scale=1.0 / math.sqrt(D))
                for half in range(2):
                    kb = 2 * p + half
                    m = kb - 4 * qc
                    if m >= 0:
                        mo = 384 - 128 * m
                        nc.vector.tensor_mul(
                            pt[:, half * TQ:(half + 1) * TQ],
                            pt[:, half * TQ:(half + 1) * TQ],
                            mask[:, mo:mo + TQ])
                pt_of[idx] = pt

            def emit_pv(idx):
                h, qc, p, npair = work[idx]
                ht, hp = h // 2, (h % 2) * D
                if p == 0:
                    y_of[(h, qc)] = ps_y.tile([D + 1, TQ], dt.float32,
                                              tag="y", name=f"y{h}_{qc}")
                y_ps = y_of[(h, qc)]
                pt = pt_of.pop(idx)
                for half in range(2):
                    kb = 2 * p + half
                    nc.tensor.matmul(
                        y_ps[:],
                        vt[kb][:, h * (D + 1):(h + 1) * (D + 1)],
                        pt[:, half * TQ:(half + 1) * TQ],
                        start=(kb == 0), stop=(kb == 2 * npair - 1))
                if p == npair - 1:
                    rec = rc_sb.tile([1, TQ], dt.float32, tag="rec",
                                     name=f"rec{idx}")
                    nc.vector.reciprocal(rec[:], y_ps[D:D + 1, :])
                    bcast = rc_sb.tile([D, TQ], dt.float32, tag="bcast",
                                       name=f"bcast{idx}")
                    nc.gpsimd.partition_broadcast(bcast[:], rec[:])
                    ysb = rc_sb.tile([D, TQ], dt.float32, tag="ysb",
                                     name=f"ysb{idx}")
                    nc.vector.tensor_copy(ysb[:], y_ps[0:D, :])
                    nc.vector.tensor_mul(
                        yT[ht][hp:hp + D, qc * TQ:(qc + 1) * TQ],
                        ysb[:], bcast[:])

            LOOKAHEAD = 2
            for j in range(min(LOOKAHEAD, len(work))):
                emit_s(j)
            for i in range(len(work)):
                if i + LOOKAHEAD < len(work):
                    emit_s(i + LOOKAHEAD)
                emit_pv(i)


        if "d" not in stages:
            for i in range(4):
                nc.sync.dma_start(out_d[i * 128:(i + 1) * 128, :],
                                  yT[i][:, 0:C].bitcast(dt.float32))
            return

        # ============ Stage D: output projection ============
        with ExitStack() as od:
            wp_sb = od.enter_context(tc.tile_pool(name="wp", bufs=1))
            o_sb = od.enter_context(tc.tile_pool(name="osb", bufs=3))
            ps_o = od.enter_context(tc.tile_pool(name="ps_o", bufs=4, space="PSUM"))

            wp = []
            for cp in range(4):
                w = wp_sb.tile([128, C], dt.float32r, tag=f"wp{cp}")
                nc.gpsimd.dma_start(w[:], wp_d[cp * 128:(cp + 1) * 128, :])
                wp.append(w)

            for g in range(NTT):
                for oc in range(2):
                    p_o = ps_o.tile([128, 512], dt.float32)
                    for cp in range(4):
                        nc.tensor.matmul(
                            p_o[:], yT[cp][:, g * 128:(g + 1) * 128],
                            wp[cp][:, oc * 512:(oc + 1) * 512],
                            start=(cp == 0), stop=False)
                    nc.tensor.matmul(p_o[:], ones,
                                     pbrow[:, oc * 512:(oc + 1) * 512],
                                     start=False, stop=True)
                    osb = o_sb.tile([128, 512], dt.float32)
                    nc.scalar.copy(osb[:], p_o[:])
                    nc.sync.dma_start(
                        out_d[g * 128:(g + 1) * 128, oc * 512:(oc + 1) * 512],
                        osb[:])


def make_in_maps(x, W_attn, b_attn, W_proj, b_proj):
    x = np.ascontiguousarray(x, dtype=np.float32)
    W_attn = np.ascontiguousarray(W_attn, dtype=np.float32)
    b_attn = np.ascontiguousarray(b_attn, dtype=np.float32)
    W_proj = np.ascontiguousarray(W_proj, dtype=np.float32)
    b_proj = np.ascontiguousarray(b_proj, dtype=np.float32)

    # combined diagonal-block mask: [:, 384-128m : 896-128m] gives the
    # pattern "valid iff j >= i + 128*m" for m in 0..3
    i = np.arange(128)[:, None]
    u = np.arange(896)[None, :]
    mask = (u >= i + 384).astype(np.float32)
    ident = np.eye(128, dtype=np.float32)

    in_maps = []
    for c in range(N_CORES):
        b, hg = divmod(c, 2)
        lo, hi = hg * S, (hg + 1) * S
        wqkv = np.concatenate(
            [W_attn[:, lo:hi], W_attn[:, C + lo:C + hi],
             W_attn[:, 2 * C + lo:2 * C + hi]], axis=1)
        # per-partition bias for the 8 q/k W-column tiles: [128, 8]
        bqk = np.stack(
            [b_attn[lo + t * 128:lo + (t + 1) * 128] for t in range(4)]
            + [b_attn[C + lo + t * 128:C + lo + (t + 1) * 128] for t in range(4)],
            axis=1)
        vbrow = b_attn[2 * C + lo:2 * C + hi][None, :]
        pbrow = (b_proj if hg == 0 else np.zeros_like(b_proj))[None, :]
        in_maps.append({
            "x": x[b],
            "wqkv": np.ascontiguousarray(wqkv),
            "bqk": np.ascontiguousarray(bqk),
            "vbrow": np.ascontiguousarray(vbrow),
            "wproj": np.ascontiguousarray(W_proj[lo:hi, :]),
            "pbrow": np.ascontiguousarray(pbrow),
            "mask": mask,
            "ident": ident,
            "onesm": np.ones((128, 128), dtype=np.float32),
        })
    return in_maps


_NC_CACHE = {}


def _get_nc(reps: int = 1, stages: str = "abcd"):
    key = (reps, stages)
    if key not in _NC_CACHE:
        nc = build_nc(reps, stages)
        nc.finalize()
        _NC_CACHE[key] = nc
    return _NC_CACHE[key]


def kernel(x, W_attn, b_attn, W_proj, b_proj):
    in_maps = make_in_maps(x, W_attn, b_attn, W_proj, b_proj)
    nc = _get_nc(1)
    res = run_bass_kernel_spmd(nc, in_maps, list(range(N_CORES)))
    out = np.empty((B, T, C), dtype=np.float32)
    for b in range(B):
        out[b] = res.results[2 * b]["out"] + res.results[2 * b + 1]["out"]
    return out

